# revision 1
# baseline (speedup 1.0000x reference)
"""Trainium2 Bass kernel for Adapt_CSA — v3 (redesigned).

Reference computation (per sample, x: (C=256, H=64, W=64) f32):
  y    = mean(x, (H,W))                       # (C,)
  y'   = conv1d(y, w_c, SAME, k=5)            # (C,)
  yc   = sigmoid(x * y'[:, None, None])       # (C, H, W)
  avg  = mean(yc, C); mx = max(yc, C)         # (H, W) each
  sa   = sigmoid(conv2d([avg, mx], w_s, SAME))# (1, H, W)
  out  = yc * sa + x

Data parallel over batch: 32 samples -> 8 cores x 4 samples.
Channel-partition layout: 2 tiles of (128, 4096) per sample, bf16.

v3 vs baseline (150.6us):
- channel-max via ONE gpsimd.partition_all_reduce (Pool engine, attn
  ucode library) instead of 33 PE transposes + 5 DVE reduces/sample;
  Pool was idle before, PE/DVE were the bottleneck
- channel-mean via 16 PE ones-matmuls (accumulating both tiles)
- patches read with 2 DMAs (3-dim window APs) instead of 10
- pipeline split into 5 skewed stages (H, E, M1, M2, T)
- measured op costs (HW): ACT pass 3.78us, DVE TT 2x 2.28us, AR 13.6us,
  TS+accum 4.4us; scalar_tensor_tensor is 1x -> plain TT used
"""

import sys

import numpy as np

sys.path.insert(0, "/opt/trn_rl_repo")

B, C, H, W = 32, 256, 64, 64
HW = H * W  # 4096
N_CORES = 8
SPC = B // N_CORES  # samples per core = 4
PAD = 68  # 64 + 2*2 zero border for SAME 5x5 conv
PADHW = PAD * PAD  # 4624
WIN = 63 * PAD + 64  # 4348 window length per patch row

_cache = {}


def _build_graph():
    import concourse.bass as bass
    import concourse.bacc as bacc
    import concourse.tile as tile
    from concourse import bass_isa, library_config, mybir

    f32 = mybir.dt.float32
    bf16 = mybir.dt.bfloat16
    AF = mybir.ActivationFunctionType
    ALU = mybir.AluOpType

    nc = bacc.Bacc("TRN2", target_bir_lowering=False)

    x_ext = nc.declare_dram_parameter("x", [SPC, 2, 128, HW], bf16, isOutput=False)
    bmat_ext = nc.declare_dram_parameter("bmat", [2, 128, 256], f32, isOutput=False)
    ws_ext = nc.declare_dram_parameter("ws", [50, 1], f32, isOutput=False)
    out_ext = nc.declare_dram_parameter("out", [SPC, 2, 128, HW], bf16, isOutput=True)

    # double-buffered padded avg/max maps + sa staging in DRAM (separate
    # tensors so tensor-granular dep tracking cannot serialize buffers)
    pads_b = [nc.dram_tensor(f"pads{j}", [2, PAD, PAD], bf16) for j in range(2)]
    salin_b = [nc.dram_tensor(f"salin{j}", [HW], bf16) for j in range(2)]

    with tile.TileContext(nc) as tc:
        with (
            tc.tile_pool(name="singles", bufs=1) as singles,
            tc.tile_pool(name="px", bufs=4) as px,
            tc.tile_pool(name="pyc", bufs=4) as pyc,
            tc.tile_pool(name="pjunk", bufs=1) as pjunk,
            tc.tile_pool(name="ppm", bufs=2) as ppm,
            tc.tile_pool(name="parm", bufs=1) as parm,
            tc.tile_pool(name="ppatch", bufs=1) as ppatch,
            tc.tile_pool(name="psar", bufs=2) as psar,
            tc.tile_pool(name="psab", bufs=2) as psab,
            tc.tile_pool(name="pmst", bufs=1) as pmst,
            tc.tile_pool(name="small", bufs=4) as small,
            tc.tile_pool(name="ps_y", bufs=2, space="PSUM") as ps_y,
            tc.tile_pool(name="ps_mean", bufs=1, space="PSUM") as ps_mean,
            tc.tile_pool(name="ps_sa", bufs=1, space="PSUM") as ps_sa,
        ):
            nc.gpsimd.load_library(library_config.attn)

            # ---- constants (issued off the SP queue so the first x load
            # is SP's first instruction) ----
            bmat_sb = singles.tile([128, 2, 256], f32)
            nc.scalar.dma_start(
                out=bmat_sb, in_=bmat_ext[:].rearrange("t p m -> p t m")
            )
            ws_f32 = singles.tile([50, 1], f32)
            nc.scalar.dma_start(out=ws_f32, in_=ws_ext[:])
            ws_bf = singles.tile([50, 1], bf16)
            nc.vector.tensor_copy(out=ws_bf, in_=ws_f32)
            ones_bf = singles.tile([128, 1], bf16)
            nc.vector.memset(ones_bf, 1.0)
            # zero both DRAM pad buffers once (interiors get overwritten
            # every sample; borders stay zero)
            zero68 = singles.tile([PAD, 2, PAD], bf16)
            nc.vector.memset(zero68, 0.0)
            for j in range(2):
                nc.gpsimd.dma_start(
                    out=pads_b[j][:].rearrange("c h w -> h c w"), in_=zero68
                )
            junk0 = pjunk.tile([128, HW], bf16)
            junk1 = pjunk.tile([128, HW], bf16)

            # per-sample state carried between stages
            st = [dict() for _ in range(SPC)]

            def stage_Hload(s):
                x_t = px.tile([128, 2, HW], bf16, tag="x")
                for t in range(2):
                    nc.sync.dma_start(out=x_t[:, t], in_=x_ext[s, t])
                st[s].update(x_t=x_t)

            def stage_Hgap(s):
                x_t = st[s]["x_t"]
                ysum = small.tile([128, 2], f32, tag="ysum")
                nc.scalar.activation(
                    out=junk0,
                    in_=x_t[:, 0],
                    func=AF.Copy,
                    bias=0.0,
                    scale=1.0,
                    accum_out=ysum[:, 0:1],
                )
                nc.vector.tensor_scalar(
                    out=junk1,
                    in0=x_t[:, 1],
                    scalar1=1.0,
                    scalar2=0.0,
                    op0=ALU.mult,
                    op1=ALU.add,
                    accum_out=ysum[:, 1:2],
                )
                st[s].update(ysum=ysum)

            def stage_E(s):
                x_t, ysum = st[s]["x_t"], st[s]["ysum"]
                py_t = ps_y.tile([128, 2], f32, tag="py")
                for mt in range(2):
                    for kt in range(2):
                        nc.tensor.matmul(
                            py_t[:, mt : mt + 1],
                            lhsT=bmat_sb[:, kt, mt * 128 : (mt + 1) * 128],
                            rhs=ysum[:, kt : kt + 1],
                            start=(kt == 0),
                            stop=(kt == 1),
                        )
                yscale = small.tile([128, 2], f32, tag="yscale")
                nc.scalar.copy(out=yscale, in_=py_t)
                yc = pyc.tile([128, 2, HW], bf16, tag="yc")
                for t in range(2):
                    nc.scalar.activation(
                        out=yc[:, t],
                        in_=x_t[:, t],
                        func=AF.Sigmoid,
                        scale=yscale[:, t : t + 1],
                    )
                st[s].update(yc=yc)

            def stage_M1(s):
                pbuf = s % 2
                yc = st[s]["yc"]

                # pre-max over the two channel tiles (DVE, 2x)
                pm = ppm.tile([128, HW], bf16, tag="pm")
                nc.vector.tensor_max(out=pm, in0=yc[:, 0], in1=yc[:, 1])

                # channel max across 128 partitions (Pool, attn library)
                armax = parm.tile([128, HW], bf16, tag="armax")
                nc.gpsimd.partition_all_reduce(
                    armax[:], pm[:], 128, bass_isa.ReduceOp.max
                )
                nc.gpsimd.dma_start(
                    out=pads_b[pbuf][1, 2:66, 2:66],
                    in_=armax[0:1, :].rearrange("p (h w) -> p h w", h=64),
                )

                # channel mean: 16 ones-matmuls (acc. both tiles) ->
                # psum rows {0,32,64,96}
                pmean = ps_mean.tile([128, 1024], f32, tag="mean")
                for k in range(4):
                    for h in range(2):
                        c0 = 1024 * k + 512 * h
                        for t in range(2):
                            nc.tensor.matmul(
                                pmean[32 * k : 32 * k + 1, 512 * h : 512 * (h + 1)],
                                lhsT=ones_bf,
                                rhs=yc[:, t, c0 : c0 + 512],
                                start=(t == 0),
                                stop=(t == 1),
                                tile_position=(0, 32 * k),
                            )
                st[s].update(pmean=pmean)

            def stage_M1b(s):
                pbuf = s % 2
                pmean = st[s]["pmean"]
                mstage = pmst.tile([128, 1024], bf16, tag="mstage")
                nc.scalar.copy(out=mstage, in_=pmean)
                for k in range(4):
                    nc.sync.dma_start(
                        out=pads_b[pbuf][0, 2 + 16 * k : 2 + 16 * (k + 1), 2:66],
                        in_=mstage[32 * k : 32 * k + 1, :].rearrange(
                            "p (hh w) -> p hh w", hh=16
                        ),
                    )

                # patches from DRAM pads: 50 rows (c,ky,kx), pitch 68
                patches = ppatch.tile([50, 4352], bf16, tag="patch")
                for c in range(2):
                    src = bass.AP(
                        tensor=pads_b[pbuf],
                        offset=c * PADHW,
                        ap=[[PAD, 5], [1, 5], [1, WIN]],
                    )
                    nc.sync.dma_start(
                        out=patches[25 * c : 25 * (c + 1), 0:WIN], in_=src
                    )
                st[s].update(patches=patches)

            def stage_M2(s):
                pbuf = s % 2
                patches = st[s]["patches"]
                # 5x5 conv as matmul, psum rows {0,32,64,96}, pitch 1088
                psa = ps_sa.tile([128, 1088], f32, tag="sa")
                for k in range(4):
                    for c0, nn in ((0, 512), (512, 512), (1024, 64)):
                        nc.tensor.matmul(
                            psa[32 * k : 32 * k + 1, c0 : c0 + nn],
                            lhsT=ws_bf,
                            rhs=patches[:, 1088 * k + c0 : 1088 * k + c0 + nn],
                            start=True,
                            stop=True,
                            tile_position=(0, 32 * k),
                        )
                sa_row = psar.tile([128, 1088], bf16, tag="sarow")
                nc.scalar.activation(out=sa_row, in_=psa, func=AF.Sigmoid)

                # sa rows (68-pitch) -> linear DRAM staging, then a
                # stride-0 broadcast read back to [128, 4096]; the last
                # sample broadcasts per-window so the tail fuse overlaps
                sab = psab.tile([128, HW], bf16, tag="sab")
                # in the windowed tail, keep SP free for out-stores:
                # salin writes issue from ACT, bcast reads from Pool
                weng = nc.scalar if s >= SPC - 2 else nc.sync
                beng = nc.gpsimd if s >= SPC - 2 else nc.sync
                for k in range(4):
                    weng.dma_start(
                        out=salin_b[pbuf][1024 * k : 1024 * (k + 1)].rearrange(
                            "(p hh w) -> p hh w", p=1, hh=16
                        ),
                        in_=sa_row[32 * k : 32 * k + 1, :].rearrange(
                            "p (hh w) -> p hh w", hh=16
                        )[:, :, 0:64],
                    )
                    if s >= SPC - 2:
                        beng.dma_start(
                            out=sab[:, 1024 * k : 1024 * (k + 1)],
                            in_=bass.AP(
                                tensor=salin_b[pbuf],
                                offset=1024 * k,
                                ap=[[0, 128], [1, 1024]],
                            ),
                        )
                if s < SPC - 2:
                    nc.sync.dma_start(
                        out=sab,
                        in_=bass.AP(
                            tensor=salin_b[pbuf], offset=0, ap=[[0, 128], [1, HW]]
                        ),
                    )
                st[s].update(sab=sab)

            def stage_T(s):
                x_t, yc, sab = st[s]["x_t"], st[s]["yc"], st[s]["sab"]
                if s >= SPC - 2:
                    # tail: per-1024-px windows so fuse/out overlap bcast
                    for k in range(4):
                        c0, c1 = 1024 * k, 1024 * (k + 1)
                        for t in range(2):
                            nc.vector.tensor_mul(
                                out=yc[:, t, c0:c1],
                                in0=yc[:, t, c0:c1],
                                in1=sab[:, c0:c1],
                            )
                            nc.vector.tensor_add(
                                out=x_t[:, t, c0:c1],
                                in0=yc[:, t, c0:c1],
                                in1=x_t[:, t, c0:c1],
                            )
                            nc.sync.dma_start(
                                out=out_ext[s, t, :, c0:c1], in_=x_t[:, t, c0:c1]
                            )
                else:
                    for t in range(2):
                        nc.vector.tensor_mul(out=yc[:, t], in0=yc[:, t], in1=sab)
                        nc.vector.tensor_add(
                            out=x_t[:, t], in0=yc[:, t], in1=x_t[:, t]
                        )
                        nc.sync.dma_start(out=out_ext[s, t], in_=x_t[:, t])

            # Emission order per step tuned per engine:
            # - x load first (SP's head, longest DMA)
            # - E(s-1): ACT sigmoids + PE chconv (inputs ready)
            # - M2(s-3): PE conv early, ACT sa_row 2nd, salin/bcast DMAs
            # - M1a(s-2): DVE pre-max early (unblocks Pool AR), PE mean
            # - T(s-4): fuse (sab landed last step)
            # - Hgap(s): GAP streams (x load in flight)
            # - M1b(s-2): mstage copy last on ACT (waits PE mean), pads+patches
            stages = (
                (0, stage_Hload),
                (1, stage_E),
                (3, stage_M2),
                (2, stage_M1),
                (2, stage_M1b),
                (4, stage_T),
                (0, stage_Hgap),
            )
            for step in range(SPC + 4):
                for d, fn in stages:
                    s = step - d
                    if 0 <= s < SPC:
                        fn(s)

    nc.compile()
    return nc


def _prep_inputs(x, w_c, w_s):
    """Shard + build per-core input maps (host side, cheap)."""
    import ml_dtypes

    wc = np.asarray(w_c, dtype=np.float32).reshape(5)
    ws4 = np.asarray(w_s, dtype=np.float32).reshape(2, 5, 5)

    # banded matrix: y'[m] = sum_k y[k] * wc[k - m + 2];  GAP 1/4096 folded in
    k = np.arange(C)[:, None]
    m = np.arange(C)[None, :]
    d = k - m + 2
    bmat = np.where((d >= 0) & (d < 5), wc[np.clip(d, 0, 4)], 0.0).astype(np.float32)
    bmat = (bmat / HW).reshape(2, 128, 256)

    # conv weights vector, rows = c*25 + ky*5 + kx ; channel-mean 1/256 folded in
    wsv = ws4.copy()
    wsv[0] /= C
    wsv = wsv.reshape(50, 1).astype(np.float32)

    xs = np.asarray(x, dtype=np.float32).astype(ml_dtypes.bfloat16).reshape(
        N_CORES, SPC, 2, 128, HW
    )
    in_maps = [{"x": xs[i], "bmat": bmat, "ws": wsv} for i in range(N_CORES)]
    return in_maps


def run(x, w_c, w_s, trace=False):
    from concourse.bass_utils import run_bass_kernel_spmd

    if "nc" not in _cache:
        _cache["nc"] = _build_graph()
    nc = _cache["nc"]
    in_maps = _prep_inputs(x, w_c, w_s)
    res = run_bass_kernel_spmd(
        nc, in_maps, core_ids=list(range(N_CORES)), trace=trace
    )
    out = np.concatenate(
        [
            res.results[i]["out"].astype(np.float32).reshape(SPC, C, H, W)
            for i in range(N_CORES)
        ],
        axis=0,
    )
    return out, res


def kernel(x, w_c, w_s):
    out, _ = run(x, w_c, w_s, trace=False)
    return out.astype(np.float32)

